# revision 106
# baseline (speedup 1.0000x reference)
"""Multi-head attention (B=2, S=2048, D=1024, H=16, Dh=64) on 8 TRN2 cores.

Sharding: data-parallel over batch (2) x tensor-parallel over heads (16 -> 4
groups of 4). Core c handles batch c//4, heads [4*(c%4), 4*(c%4)+4).
Each core computes its partial output projection (Wo column slice); the host
sums the 4 per-core partials per batch (the "all-reduce") and adds bo.

v3 design (cost-model driven; PE cost = sum of matmul moving dims only):
  - Scores in the exp2 domain (log2e/sqrt(dh) folded into Wq); exp runs on
    ACT as Exp(scale=ln2) except every POOL_EVERY-th tile, which DVE stages
    PSUM->SBUF and GPSIMD computes as pow(2, x) -> fp16.
  - attn@V is d-moving: oacc[q, 65] += ex[k, qb].T @ (v|ones)[k, 65]; the
    ones column accumulates the softmax denominator. Normalization is a
    per-partition reciprocal multiply on DVE. fp16 [q, j] stage tiles are
    transposed to [j, q] by DMA xbar transposes for the output projection.
  - V is projected directly in [s, j] (lhsT = x_v s-block), no transposes.
  - Slot machine: task i's scores/exp run one task AHEAD of task i-1's
    attn@V, hiding the whole exp latency chain; Q/K projection tiles (by
    (jb, stl)) and V s-blocks are just-in-time filler quanta with deadlines,
    drained EDF into the PE slack of each slot. x streams stl-granular so
    the first head starts after only 3 projection tiles.
"""

import numpy as np
from contextlib import ExitStack

import concourse.bass as bass
from concourse import bacc
import concourse.mybir as mybir
import concourse.tile as tile

F32 = mybir.dt.float32
F16 = mybir.dt.float16
AF = mybir.ActivationFunctionType
ALU = mybir.AluOpType

B = 2
S = 2048
D = 1024
H = 16
DH = 64
NCORES = 8
HL = 4          # heads per core
J = HL * DH     # 256 local projection width
P = 128
KD = D // P     # 8 d-chunks (contraction steps)
KB = S // P     # 16 k-blocks
QW = 1024       # q columns per attention task
EB = D // P     # 8 e-blocks
LN2 = float(np.log(2.0))

POOL_EVERY = 5  # every POOL_EVERY-th exp tile -> DVE stage + gpsimd pow2
DEBUG_TAPS = False


def build_nc():
    nc = bacc.Bacc()

    xq = nc.dram_tensor("xq", [P, KD, S], F16, kind="ExternalInput")
    xk = nc.dram_tensor("xk", [P, KD, S], F16, kind="ExternalInput")
    xv = nc.dram_tensor("xv", [P, 4, 4 * KD * P], F16, kind="ExternalInput")
    wq = nc.dram_tensor("wq", [P, KD, J], F16, kind="ExternalInput")
    wk = nc.dram_tensor("wk", [P, KD, J], F16, kind="ExternalInput")
    wv = nc.dram_tensor("wv", [P, KD, J], F16, kind="ExternalInput")
    wo = nc.dram_tensor("wo", [P, 2, D], F16, kind="ExternalInput")
    out_t = nc.dram_tensor("out_t", [EB, P, S], F16, kind="ExternalOutput")
    if DEBUG_TAPS:
        dbg_qt = nc.dram_tensor("dbg_qt", [P, 2, S], F16, kind="ExternalOutput")
        dbg_kt = nc.dram_tensor("dbg_kt", [P, 2, S], F16, kind="ExternalOutput")
        dbg_v = nc.dram_tensor("dbg_v", [P, KB, HL, DH + 1], F16,
                               kind="ExternalOutput")
        dbg_ao = nc.dram_tensor("dbg_ao", [P, 2, S], F16, kind="ExternalOutput")

    with tile.TileContext(nc) as tc, ExitStack() as st:
        const = st.enter_context(tc.tile_pool(name="const", bufs=1))
        persist = st.enter_context(tc.tile_pool(name="persist", bufs=1))
        xvpool = st.enter_context(tc.tile_pool(name="xvp", bufs=2))

        wq_sb = const.tile([P, KD, J], F16, tag="wq")
        wk_sb = const.tile([P, KD, J], F16, tag="wk")
        wv_sb = const.tile([P, KD, J], F16, tag="wv")
        wo_sb = const.tile([P, 2, D], F16, tag="wo")
        base2 = const.tile([P, QW], F32, tag="base2")

        xq_sb = persist.tile([P, KD, S], F16, tag="xq")
        xk_sb = persist.tile([P, KD, S], F16, tag="xk")
        qt_sb = persist.tile([P, 2, S], F16, tag="qt")    # Q_T [256, 2048]
        kt_sb = persist.tile([P, 2, S], F16, tag="kt")    # K_T
        v_sb = persist.tile([P, KB, HL, DH + 1], F16, tag="v")  # V + ones
        ao_js = persist.tile([P, 2, S], F16, tag="ao")    # [j, q] for oproj

        # ---- DMA preamble (SP queue, stl-granular so JIT proj can start) --
        def dma_x(dst, src, stl, dchalf=None):
            d0, d1 = (0, KD) if dchalf is None else (dchalf * 4, dchalf * 4 + 4)
            nc.sync.dma_start(
                out=dst[:, d0:d1, stl * 512:(stl + 1) * 512],
                in_=src[:, d0:d1, stl * 512:(stl + 1) * 512])

        nc.sync.dma_start(out=wk_sb[:], in_=wk[:])
        dma_x(xk_sb, xk, 0)
        nc.sync.dma_start(out=wq_sb[:], in_=wq[:])
        dma_x(xq_sb, xq, 0)
        dma_x(xq_sb, xq, 1)
        dma_x(xk_sb, xk, 1)
        dma_x(xk_sb, xk, 2)
        dma_x(xk_sb, xk, 3)
        nc.sync.dma_start(out=wv_sb[:], in_=wv[:])
        xv_ch = []
        for g in range(4):
            c = xvpool.tile([P, 4, KD * P], F16, tag="xvc", name=f"xv{g}")
            nc.sync.dma_start(out=c[:], in_=xv[:, g, :])
            xv_ch.append(c)
        dma_x(xq_sb, xq, 2)
        dma_x(xq_sb, xq, 3)
        nc.sync.dma_start(out=wo_sb[:], in_=wo[:])
        nc.gpsimd.memset(base2[:], 2.0)
        nc.gpsimd.memset(v_sb[:, :, :, DH:DH + 1], 1.0)

        # ---- helpers ------------------------------------------------------
        with tc.tile_pool(name="psc", bufs=4, space="PSUM") as psc, \
             tc.tile_pool(name="poacc", bufs=2, space="PSUM") as poacc, \
             tc.tile_pool(name="paux", bufs=2, space="PSUM") as paux, \
             tc.tile_pool(name="expp", bufs=36) as expp, \
             tc.tile_pool(name="stgp", bufs=3) as stgp, \
             tc.tile_pool(name="aost", bufs=3) as aop, \
             tc.tile_pool(name="npool", bufs=4) as npool, \
             tc.tile_pool(name="ostage", bufs=6) as opool:

            # Q/K projection tile (tensor, jb, stl): PSUM [128 j, 512 s],
            # accumulated over 8 dc in 2 quanta of 4, evacuated to qt/kt.
            def make_proj_tile(xsb, wsb, dst, jb, stl, name):
                state = {"ps": None}

                def quantum(q):
                    if q == 0:
                        state["ps"] = paux.tile([P, 512], F32, tag="vp",
                                                name=f"pp_{name}")
                    ps = state["ps"]
                    for dc in range(q * 4, q * 4 + 4):
                        nc.tensor.matmul(
                            ps[:],
                            wsb[:, dc, jb * P:(jb + 1) * P],
                            xsb[:, dc, stl * 512:(stl + 1) * 512],
                            start=(dc == 0),
                            stop=(dc == KD - 1),
                        )
                    if q == 1:
                        nc.vector.tensor_copy(
                            dst[:, jb, stl * 512:(stl + 1) * 512], ps[:])

                return quantum

            # V s-block job: PSUM [128 s, 256 j] over 8 dc, one quantum.
            def vjob(sb):
                vp = paux.tile([P, J], F32, tag="vp", name=f"vp{sb}")
                xc = xv_ch[sb // 4]
                for dc in range(KD):
                    nc.tensor.matmul(
                        vp[:],
                        xc[:, sb % 4, dc * P:(dc + 1) * P],
                        wv_sb[:, dc, :],
                        start=(dc == 0),
                        stop=(dc == KD - 1),
                    )
                nc.vector.tensor_copy(v_sb[:, sb, :, 0:DH], vp[:])

            # oproj unit: 256 out cols, jb0+jb1 accumulate, fp16 stage+DMA.
            ob_group = {}

            def oproj_unit(qh, eb, u, pool=None, act_evac=False):
                q0 = qh * QW
                s0 = q0 + u * 256
                pool = pool if pool is not None else paux
                tag = "vp" if pool is paux else "oacc"
                po = pool.tile([P, J], F32, tag=tag, name=f"po{qh}_{eb}_{u}")
                for jb in range(2):
                    nc.tensor.matmul(
                        po[:, 0:256],
                        wo_sb[:, jb, eb * P:(eb + 1) * P],
                        ao_js[:, jb, s0:s0 + 256],
                        start=(jb == 0),
                        stop=(jb == 1),
                    )
                key = (qh, eb)
                if key not in ob_group:
                    ob_group[key] = opool.tile([P, QW], F16, tag="ob",
                                               name=f"ob{qh}_{eb}")
                ob = ob_group[key]
                if qh == 1 or act_evac:
                    nc.scalar.copy(ob[:, u * 256:(u + 1) * 256], po[:, 0:256])
                else:
                    nc.vector.tensor_copy(ob[:, u * 256:(u + 1) * 256],
                                          po[:, 0:256])
                if u in (1, 3):
                    h0 = (u - 1) * 256
                    nc.sync.dma_start(out=out_t[eb][:, q0 + h0:q0 + h0 + 512],
                                      in_=ob[:, h0:h0 + 512])

            # split-jb variant for the last q-half: jb0 partials run during
            # task 7 (only needs transpose(1,0)); the jb1 pass + add is the
            # short tail gated on the final norm.
            def oproj_half(qh, eb, u, jb):
                q0 = qh * QW
                s0 = q0 + u * 256
                po = paux.tile([P, J], F32, tag="vp",
                               name=f"ph{qh}_{eb}_{u}_{jb}")
                nc.tensor.matmul(
                    po[:, 0:256],
                    wo_sb[:, jb, eb * P:(eb + 1) * P],
                    ao_js[:, jb, s0:s0 + 256],
                    start=True,
                    stop=True,
                )
                key = (qh, eb)
                if key not in ob_group:
                    ob_group[key] = opool.tile([P, QW], F16, tag="ob",
                                               name=f"ob{qh}_{eb}")
                ob = ob_group[key]
                dst = ob[:, u * 256:(u + 1) * 256]
                if jb == 0:
                    # runs inside task 7 where ACT is still on exp: use DVE
                    nc.vector.tensor_copy(dst, po[:, 0:256])
                else:
                    nc.vector.tensor_add(dst, dst, po[:, 0:256])
                    if u in (1, 3):
                        h0 = (u - 1) * 256
                        nc.sync.dma_start(
                            out=out_t[eb][:, q0 + h0:q0 + h0 + 512],
                            in_=ob[:, h0:h0 + 512])

            ao_stage = {}

            def normalize(ti, oaccs, transp_cb=None):
                qh, h = TASKS[ti]
                jb = h // 2
                off = DH * (h % 2)
                key = (qh, jb)
                if key not in ao_stage:
                    ao_stage[key] = aop.tile([P, 8, P], F16, tag="aos",
                                             name=f"aos{qh}_{jb}")
                aos = ao_stage[key]
                for g in range(2):
                    rc = npool.tile([P, 4], F32, tag="rc")
                    with tc.high_priority(offset=400):
                        nc.vector.reciprocal_approx_fast(
                            rc[:], oaccs[g][:, DH::DH + 1])
                    for q4 in range(4):
                        qb = g * 4 + q4
                        src_ap = oaccs[g][:, q4 * (DH + 1):q4 * (DH + 1) + DH]
                        dst_ap = aos[:, qb, off:off + DH]
                        nc.vector.tensor_scalar(
                            dst_ap, src_ap, rc[:, q4:q4 + 1], None,
                            op0=ALU.mult)
                    if transp_cb is not None:
                        transp_cb(g)  # transpose qb-group as soon as normed

            def transpose_jb(qh, jb, qbs=range(8)):
                aos = ao_stage[(qh, jb)]
                q0 = qh * QW
                for qb in qbs:
                    nc.sync.dma_start_transpose(
                        ao_js[:, jb, q0 + qb * P:q0 + (qb + 1) * P],
                        aos[:, qb, :],
                    )

            # ---- filler queue: (deadline_slot, avail_slot, closure) -------
            filler = []

            def add_proj(tensor_sb, wsb, dst, jb, stl, dl, name):
                qf = make_proj_tile(tensor_sb, wsb, dst, jb, stl, name)
                filler.append([dl - 1, 0, lambda: qf(0)])
                filler.append([dl, 0, lambda: qf(1)])

            for s4 in range(1, 4):     # K jb0 stl 1..3, before task0 kb 4s
                add_proj(xk_sb, wk_sb, kt_sb, 0, s4, 4 * s4, f"k0{s4}")
            for k in range(KB):        # vjob k before av(0,k) at slot 16+k
                filler.append([16 + k, 0, (lambda kk: lambda: vjob(kk))(k)])
            for s4 in range(4):        # K jb1 before task2 kb 4s
                add_proj(xk_sb, wk_sb, kt_sb, 1, s4, 32 + 4 * s4, f"k1{s4}")
            for s4 in range(2):        # Q jb1 stl 0,1 before task 2
                add_proj(xq_sb, wq_sb, qt_sb, 1, s4, 31, f"q1{s4}")
            for s4 in range(2, 4):     # Q jb0 stl 2,3 before task 4
                add_proj(xq_sb, wq_sb, qt_sb, 0, s4, 63, f"q0{s4}")
            for s4 in range(2, 4):     # Q jb1 stl 2,3 before task 6
                add_proj(xq_sb, wq_sb, qt_sb, 1, s4, 95, f"q1{s4}")
            for eb in range(EB):       # oproj qh0 after transpose(0,1)@80
                for u in range(4):
                    filler.append([126, 81,
                                   (lambda e, uu: lambda: oproj_unit(0, e, uu))(eb, u)])
            for eb in range(4):        # oproj qh1 jb0-pass after transp(1,0)
                for u in range(4):     # (only 4 ebs: ostage pool depth)
                    filler.append([127, 113,
                                   (lambda e, uu: lambda: oproj_half(1, e, uu, 0))(eb, u)])

            filler.sort(key=lambda f: f[0])

            def drain_filler(slot, budget):
                done = 0
                for f in filler:
                    if done >= budget:
                        break
                    if f[1] <= slot:
                        f[2]()
                        f[0] = -1000
                        done += 1
                filler[:] = [f for f in filler if f[0] != -1000]
                return done

            # ---- phase 1: Q jb0 stl0/1 + K jb0 stl0 -----------------------
            for nm, (xsb, wsb, dst, jb, stl) in [
                ("ph_k00", (xk_sb, wk_sb, kt_sb, 0, 0)),
                ("ph_q00", (xq_sb, wq_sb, qt_sb, 0, 0)),
                ("ph_q01", (xq_sb, wq_sb, qt_sb, 0, 1)),
            ]:
                qf = make_proj_tile(xsb, wsb, dst, jb, stl, nm)
                qf(0)
                qf(1)

            # ---- slot machine ---------------------------------------------
            TASKS = [(qh, h) for qh in range(2) for h in range(HL)]
            oacc_of = {}

            def emit_scores_exp(ti, kb, n):
                qh, h = TASKS[ti]
                q0 = qh * QW + n * 512
                jb = h // 2
                off = DH * (h % 2)
                sc = psc.tile([P, 512], F32, tag="sc")
                nc.tensor.matmul(
                    sc[:],
                    kt_sb[off:off + DH, jb, kb * P:(kb + 1) * P],
                    qt_sb[off:off + DH, jb, q0:q0 + 512],
                    start=True,
                    stop=True,
                )
                ex = expp.tile([P, 512], F16, tag="ex", name=f"ex{ti}_{kb}_{n}")
                if (kb * 2 + n) % POOL_EVERY == 1 and kb < KB - 1:
                    stg = stgp.tile([P, 512], F32, tag="stg")
                    nc.vector.tensor_copy(stg[:], sc[:])
                    nc.gpsimd.tensor_tensor(ex[:], base2[:, 0:512],
                                            stg[:], op=ALU.pow)
                else:
                    nc.scalar.activation(ex[:], sc[:], AF.Exp, scale=LN2)
                return ex

            def emit_attnv(ti, kb, n, ex):
                qh, h = TASKS[ti]
                oacc = oacc_of[ti][n]
                for q4 in range(4):
                    # start=False always: start=True clears the whole PSUM
                    # bank, destroying the other q4 regions' partial sums.
                    # The bank is zeroed once by memzero at tile alloc.
                    nc.tensor.matmul(
                        oacc[:, q4 * (DH + 1):(q4 + 1) * (DH + 1)],
                        ex[:, q4 * P:(q4 + 1) * P],
                        v_sb[:, kb, h, :],
                        start=False,
                        stop=(kb == KB - 1),
                        skip_group_check=True,
                    )

            ex_of = {}
            for ti in range(8):
                # task top: norm(ti-2), transposes, oacc alloc
                if ti >= 2:
                    normalize(ti - 2, oacc_of[ti - 2])
                if ti == 3:
                    transpose_jb(0, 0)
                elif ti == 5:
                    transpose_jb(0, 1)
                elif ti == 7:
                    transpose_jb(1, 0)
                oacc_of[ti] = [
                    poacc.tile([P, 4 * (DH + 1)], F32, tag="oacc",
                               name=f"oa{ti}_{g}")
                    for g in range(2)
                ]
                for g in range(2):
                    nc.vector.memzero(oacc_of[ti][g][:])
                for kb in range(KB):
                    slot = ti * KB + kb
                    # filler and attnV (no psc dependency) first, so psc
                    # backpressure on scores doesn't starve the PE
                    drain_filler(slot, 2)
                    if ti >= 1:
                        for n in range(2):
                            emit_attnv(ti - 1, kb, n, ex_of.pop((ti - 1, kb, n)))
                    if ti == 7 and kb >= 6:
                        for n in range(2):
                            emit_attnv(7, kb - 6, n, ex_of.pop((7, kb - 6, n)))
                    for n in range(2):
                        ex_of[(ti, kb, n)] = emit_scores_exp(ti, kb, n)

            # ---- tail -----------------------------------------------------
            for kb in range(KB - 6, KB):
                for n in range(2):
                    emit_attnv(7, kb, n, ex_of.pop((7, kb, n)))
            drain_filler(200, 100)
            normalize(6, oacc_of[6])
            normalize(7, oacc_of[7])
            transpose_jb(1, 1)
            # interleave the DVE-add stream (split ebs, paux psum) with a
            # pair-fused ACT-evac stream (ebs 4-7, reusing the freed psc
            # banks, one 512-wide evac per unit pair) so both evac engines
            # and psum pools drain in parallel
            def oproj_pair(eb, up):
                q0 = QW
                po = psc.tile([P, 512], F32, tag="sc", name=f"pp2_{eb}_{up}")
                for half in range(2):
                    u = up * 2 + half
                    s0 = q0 + u * 256
                    for jb in range(2):
                        nc.tensor.matmul(
                            po[:, half * 256:half * 256 + 256],
                            wo_sb[:, jb, eb * P:(eb + 1) * P],
                            ao_js[:, jb, s0:s0 + 256],
                            start=(jb == 0),
                            stop=(jb == 1),
                        )
                ob = ob_group.setdefault(
                    (1, eb), opool.tile([P, QW], F16, tag="ob",
                                        name=f"ob1_{eb}"))
                nc.scalar.copy(ob[:, up * 512:(up + 1) * 512], po[:])
                nc.sync.dma_start(
                    out=out_t[eb][:, q0 + up * 512:q0 + (up + 1) * 512],
                    in_=ob[:, up * 512:(up + 1) * 512])

            for i in range(8):
                oproj_pair(4 + i // 2, i % 2)
                oproj_half(1, i // 2, (i % 2) * 2, 1)
                oproj_half(1, i // 2, (i % 2) * 2 + 1, 1)
            if DEBUG_TAPS:
                nc.sync.dma_start(out=dbg_qt.ap(), in_=qt_sb[:])
                nc.sync.dma_start(out=dbg_kt.ap(), in_=kt_sb[:])
                nc.sync.dma_start(out=dbg_v.ap(), in_=v_sb[:])
                nc.sync.dma_start(out=dbg_ao.ap(), in_=ao_js[:])

    nc.finalize()
    return nc


_NC_CACHE = None


def _get_nc():
    global _NC_CACHE
    if _NC_CACHE is None:
        _NC_CACHE = build_nc()
    return _NC_CACHE


def make_in_maps(query, key, value, Wq, Wk, Wv, Wo):
    """Build the 8 per-core input dicts from the full tensors (p-major)."""
    query = np.asarray(query, np.float32)
    key = np.asarray(key, np.float32)
    value = np.asarray(value, np.float32)
    Wq = np.asarray(Wq, np.float32)
    Wk = np.asarray(Wk, np.float32)
    Wv = np.asarray(Wv, np.float32)
    Wo = np.asarray(Wo, np.float32)

    def pmajor(a2d, inner):  # [Drows, inner] -> [P, Drows//P, inner]
        return np.ascontiguousarray(
            a2d.reshape(-1, P, inner).transpose(1, 0, 2)
        )

    # scores in the exp2 domain: fold log2(e)/sqrt(dh) into Wq
    scale = np.float32(np.log2(np.e) / np.sqrt(DH))
    xs = {}
    for b in range(B):
        xq = pmajor(np.ascontiguousarray(query[b].T), S).astype(np.float16)
        xk = pmajor(np.ascontiguousarray(key[b].T), S).astype(np.float16)
        xv3 = pmajor(np.ascontiguousarray(value[b].T), S).astype(np.float16)
        # xv: [P, dc, s] -> [P, g, sb4, dc*128] (s-block-major, groups of 4)
        xv = np.ascontiguousarray(
            xv3.reshape(P, KD, KB, P).transpose(0, 2, 1, 3).reshape(P, 4, -1)
        )
        xs[b] = {"xq": xq, "xk": xk, "xv": xv}
    ws = {}
    for hg in range(4):
        sl = slice(hg * J, (hg + 1) * J)
        wo_t = np.ascontiguousarray(Wo[:, sl].T)  # [256, 1024]
        ws[hg] = {
            "wq": pmajor(np.ascontiguousarray(Wq[sl].T * scale), J).astype(np.float16),
            "wk": pmajor(np.ascontiguousarray(Wk[sl].T), J).astype(np.float16),
            "wv": pmajor(np.ascontiguousarray(Wv[sl].T), J).astype(np.float16),
            "wo": np.ascontiguousarray(
                wo_t.reshape(2, P, D).transpose(1, 0, 2)
            ).astype(np.float16),
        }
    in_maps = []
    for c in range(NCORES):
        b, hg = c // 4, c % 4
        m = {}
        m.update(xs[b])
        m.update(ws[hg])
        in_maps.append(m)
    return in_maps


def assemble(results, bo):
    """Sum the 4 per-core partials per batch, add bo."""
    bo = np.asarray(bo, np.float32)
    out = np.zeros((B, S, D), np.float32)
    for c in range(NCORES):
        b = c // 4
        part = results[c]["out_t"].astype(np.float32).reshape(D, S).T
        out[b] += part
    out += bo[None, None, :]
    return out


def kernel(query, key, value, Wq, Wk, Wv, Wo, bo):
    import os
    import time

    # helps recover wedged NeuronCores between runs
    os.environ.setdefault("NEURON_RT_RESET_CORES", "1")
    from concourse.bass_utils import run_bass_kernel_spmd

    nc = _get_nc()
    in_maps = make_in_maps(query, key, value, Wq, Wk, Wv, Wo)
    last_exc = None
    for attempt in range(3):
        try:
            res = run_bass_kernel_spmd(nc, in_maps, list(range(NCORES)))
            return assemble(res.results, bo)
        except Exception as e:  # transient NRT_EXEC_UNIT_UNRECOVERABLE etc.
            last_exc = e
            time.sleep(2.0)
    raise last_exc


# revision 121
# speedup vs baseline: 1.0003x; 1.0003x over previous
"""Multi-head attention (B=2, S=2048, D=1024, H=16, Dh=64) on 8 TRN2 cores.

Sharding: data-parallel over batch (2) x tensor-parallel over heads (16 -> 4
groups of 4). Core c handles batch c//4, heads [4*(c%4), 4*(c%4)+4).
Each core computes its partial output projection (Wo column slice); the host
sums the 4 per-core partials per batch (the "all-reduce") and adds bo.

v3 design (cost-model driven; PE cost = sum of matmul moving dims only):
  - Scores in the exp2 domain (log2e/sqrt(dh) folded into Wq); exp runs on
    ACT as Exp(scale=ln2) except every POOL_EVERY-th tile, which DVE stages
    PSUM->SBUF and GPSIMD computes as pow(2, x) -> fp16.
  - attn@V is d-moving: oacc[q, 65] += ex[k, qb].T @ (v|ones)[k, 65]; the
    ones column accumulates the softmax denominator. Normalization is a
    per-partition reciprocal multiply on DVE. fp16 [q, j] stage tiles are
    transposed to [j, q] by DMA xbar transposes for the output projection.
  - V is projected directly in [s, j] (lhsT = x_v s-block), no transposes.
  - Slot machine: task i's scores/exp run one task AHEAD of task i-1's
    attn@V, hiding the whole exp latency chain; Q/K projection tiles (by
    (jb, stl)) and V s-blocks are just-in-time filler quanta with deadlines,
    drained EDF into the PE slack of each slot. x streams stl-granular so
    the first head starts after only 3 projection tiles.
"""

import numpy as np
from contextlib import ExitStack

import concourse.bass as bass
from concourse import bacc
import concourse.mybir as mybir
import concourse.tile as tile

F32 = mybir.dt.float32
F16 = mybir.dt.float16
AF = mybir.ActivationFunctionType
ALU = mybir.AluOpType

B = 2
S = 2048
D = 1024
H = 16
DH = 64
NCORES = 8
HL = 4          # heads per core
J = HL * DH     # 256 local projection width
P = 128
KD = D // P     # 8 d-chunks (contraction steps)
KB = S // P     # 16 k-blocks
QW = 1024       # q columns per attention task
EB = D // P     # 8 e-blocks
LN2 = float(np.log(2.0))

POOL_EVERY = 5  # every POOL_EVERY-th exp tile -> DVE stage + gpsimd pow2
DEBUG_TAPS = False


def build_nc():
    nc = bacc.Bacc()

    xq = nc.dram_tensor("xq", [P, KD, S], F16, kind="ExternalInput")
    xk = nc.dram_tensor("xk", [P, KD, S], F16, kind="ExternalInput")
    xv = nc.dram_tensor("xv", [P, 4, 4 * KD * P], F16, kind="ExternalInput")
    wq = nc.dram_tensor("wq", [P, KD, J], F16, kind="ExternalInput")
    wk = nc.dram_tensor("wk", [P, KD, J], F16, kind="ExternalInput")
    wv = nc.dram_tensor("wv", [P, KD, J], F16, kind="ExternalInput")
    wo = nc.dram_tensor("wo", [P, 2, D], F16, kind="ExternalInput")
    out_t = nc.dram_tensor("out_t", [EB, P, S], F16, kind="ExternalOutput")
    if DEBUG_TAPS:
        dbg_qt = nc.dram_tensor("dbg_qt", [P, 2, S], F16, kind="ExternalOutput")
        dbg_kt = nc.dram_tensor("dbg_kt", [P, 2, S], F16, kind="ExternalOutput")
        dbg_v = nc.dram_tensor("dbg_v", [P, KB, HL, DH + 1], F16,
                               kind="ExternalOutput")
        dbg_ao = nc.dram_tensor("dbg_ao", [P, 2, S], F16, kind="ExternalOutput")

    with tile.TileContext(nc) as tc, ExitStack() as st:
        const = st.enter_context(tc.tile_pool(name="const", bufs=1))
        persist = st.enter_context(tc.tile_pool(name="persist", bufs=1))
        xvpool = st.enter_context(tc.tile_pool(name="xvp", bufs=3))

        wq_sb = const.tile([P, KD, J], F16, tag="wq")
        wk_sb = const.tile([P, KD, J], F16, tag="wk")
        wv_sb = const.tile([P, KD, J], F16, tag="wv")
        wo_sb = const.tile([P, 2, D], F16, tag="wo")
        base2 = const.tile([P, QW], F32, tag="base2")

        xq_sb = persist.tile([P, KD, S], F16, tag="xq")
        xk_sb = persist.tile([P, KD, S], F16, tag="xk")
        qt_sb = persist.tile([P, 2, S], F16, tag="qt")    # Q_T [256, 2048]
        kt_sb = persist.tile([P, 2, S], F16, tag="kt")    # K_T
        v_sb = persist.tile([P, KB, HL, DH + 1], F16, tag="v")  # V + ones
        ao_js = persist.tile([P, 2, S], F16, tag="ao")    # [j, q] for oproj

        # ---- DMA preamble (SP queue, stl-granular so JIT proj can start) --
        def dma_x(dst, src, stl, dchalf=None):
            d0, d1 = (0, KD) if dchalf is None else (dchalf * 4, dchalf * 4 + 4)
            nc.sync.dma_start(
                out=dst[:, d0:d1, stl * 512:(stl + 1) * 512],
                in_=src[:, d0:d1, stl * 512:(stl + 1) * 512])

        nc.sync.dma_start(out=wk_sb[:], in_=wk[:])
        dma_x(xk_sb, xk, 0)
        nc.sync.dma_start(out=wq_sb[:], in_=wq[:])
        dma_x(xq_sb, xq, 0)
        dma_x(xq_sb, xq, 1)
        dma_x(xk_sb, xk, 1)
        dma_x(xk_sb, xk, 2)
        dma_x(xk_sb, xk, 3)
        nc.sync.dma_start(out=wv_sb[:], in_=wv[:])
        xv_ch = []
        for g in range(4):
            c = xvpool.tile([P, 4, KD * P], F16, tag="xvc", name=f"xv{g}")
            nc.sync.dma_start(out=c[:], in_=xv[:, g, :])
            xv_ch.append(c)
        dma_x(xq_sb, xq, 2)
        dma_x(xq_sb, xq, 3)
        nc.sync.dma_start(out=wo_sb[:], in_=wo[:])
        nc.gpsimd.memset(base2[:], 2.0)
        nc.gpsimd.memset(v_sb[:, :, :, DH:DH + 1], 1.0)

        # ---- helpers ------------------------------------------------------
        with tc.tile_pool(name="psc", bufs=4, space="PSUM") as psc, \
             tc.tile_pool(name="poacc", bufs=2, space="PSUM") as poacc, \
             tc.tile_pool(name="paux", bufs=2, space="PSUM") as paux, \
             tc.tile_pool(name="expp", bufs=36) as expp, \
             tc.tile_pool(name="stgp", bufs=3) as stgp, \
             tc.tile_pool(name="aost", bufs=3) as aop, \
             tc.tile_pool(name="npool", bufs=4) as npool, \
             tc.tile_pool(name="ostage", bufs=6) as opool:

            # Q/K projection tile (tensor, jb, stl): PSUM [128 j, 512 s],
            # accumulated over 8 dc in 2 quanta of 4, evacuated to qt/kt.
            def make_proj_tile(xsb, wsb, dst, jb, stl, name):
                state = {"ps": None}

                def quantum(q):
                    if q == 0:
                        state["ps"] = paux.tile([P, 512], F32, tag="vp",
                                                name=f"pp_{name}")
                    ps = state["ps"]
                    for dc in range(q * 4, q * 4 + 4):
                        nc.tensor.matmul(
                            ps[:],
                            wsb[:, dc, jb * P:(jb + 1) * P],
                            xsb[:, dc, stl * 512:(stl + 1) * 512],
                            start=(dc == 0),
                            stop=(dc == KD - 1),
                        )
                    if q == 1:
                        nc.vector.tensor_copy(
                            dst[:, jb, stl * 512:(stl + 1) * 512], ps[:])

                return quantum

            # V s-block job: PSUM [128 s, 256 j] over 8 dc, one quantum.
            def vjob(sb):
                vp = paux.tile([P, J], F32, tag="vp", name=f"vp{sb}")
                xc = xv_ch[sb // 4]
                for dc in range(KD):
                    nc.tensor.matmul(
                        vp[:],
                        xc[:, sb % 4, dc * P:(dc + 1) * P],
                        wv_sb[:, dc, :],
                        start=(dc == 0),
                        stop=(dc == KD - 1),
                    )
                nc.vector.tensor_copy(v_sb[:, sb, :, 0:DH], vp[:])

            # oproj unit: 256 out cols, jb0+jb1 accumulate, fp16 stage+DMA.
            ob_group = {}

            def oproj_unit(qh, eb, u, pool=None, act_evac=False):
                q0 = qh * QW
                s0 = q0 + u * 256
                pool = pool if pool is not None else paux
                tag = "vp" if pool is paux else "oacc"
                po = pool.tile([P, J], F32, tag=tag, name=f"po{qh}_{eb}_{u}")
                for jb in range(2):
                    nc.tensor.matmul(
                        po[:, 0:256],
                        wo_sb[:, jb, eb * P:(eb + 1) * P],
                        ao_js[:, jb, s0:s0 + 256],
                        start=(jb == 0),
                        stop=(jb == 1),
                    )
                key = (qh, eb)
                if key not in ob_group:
                    ob_group[key] = opool.tile([P, QW], F16, tag="ob",
                                               name=f"ob{qh}_{eb}")
                ob = ob_group[key]
                if qh == 1 or act_evac:
                    nc.scalar.copy(ob[:, u * 256:(u + 1) * 256], po[:, 0:256])
                else:
                    nc.vector.tensor_copy(ob[:, u * 256:(u + 1) * 256],
                                          po[:, 0:256])
                if u in (1, 3):
                    h0 = (u - 1) * 256
                    nc.sync.dma_start(out=out_t[eb][:, q0 + h0:q0 + h0 + 512],
                                      in_=ob[:, h0:h0 + 512])

            # split-jb variant for the last q-half: jb0 partials run during
            # task 7 (only needs transpose(1,0)); the jb1 pass + add is the
            # short tail gated on the final norm.
            def oproj_half(qh, eb, u, jb):
                q0 = qh * QW
                s0 = q0 + u * 256
                po = paux.tile([P, J], F32, tag="vp",
                               name=f"ph{qh}_{eb}_{u}_{jb}")
                nc.tensor.matmul(
                    po[:, 0:256],
                    wo_sb[:, jb, eb * P:(eb + 1) * P],
                    ao_js[:, jb, s0:s0 + 256],
                    start=True,
                    stop=True,
                )
                key = (qh, eb)
                if key not in ob_group:
                    ob_group[key] = opool.tile([P, QW], F16, tag="ob",
                                               name=f"ob{qh}_{eb}")
                ob = ob_group[key]
                dst = ob[:, u * 256:(u + 1) * 256]
                if jb == 0:
                    # runs inside task 7 where ACT is still on exp: use DVE
                    nc.vector.tensor_copy(dst, po[:, 0:256])
                else:
                    nc.vector.tensor_add(dst, dst, po[:, 0:256])
                    if u in (1, 3):
                        h0 = (u - 1) * 256
                        nc.sync.dma_start(
                            out=out_t[eb][:, q0 + h0:q0 + h0 + 512],
                            in_=ob[:, h0:h0 + 512])

            ao_stage = {}

            def normalize(ti, oaccs, transp_cb=None):
                qh, h = TASKS[ti]
                jb = h // 2
                off = DH * (h % 2)
                key = (qh, jb)
                if key not in ao_stage:
                    ao_stage[key] = aop.tile([P, 8, P], F16, tag="aos",
                                             name=f"aos{qh}_{jb}")
                aos = ao_stage[key]
                for g in range(2):
                    rc = npool.tile([P, 4], F32, tag="rc")
                    with tc.high_priority(offset=400):
                        nc.vector.reciprocal_approx_fast(
                            rc[:], oaccs[g][:, DH::DH + 1])
                    for q4 in range(4):
                        qb = g * 4 + q4
                        src_ap = oaccs[g][:, q4 * (DH + 1):q4 * (DH + 1) + DH]
                        dst_ap = aos[:, qb, off:off + DH]
                        nc.vector.tensor_scalar(
                            dst_ap, src_ap, rc[:, q4:q4 + 1], None,
                            op0=ALU.mult)
                    if transp_cb is not None:
                        transp_cb(g)  # transpose qb-group as soon as normed

            def transpose_jb(qh, jb, qbs=range(8)):
                aos = ao_stage[(qh, jb)]
                q0 = qh * QW
                for qb in qbs:
                    nc.sync.dma_start_transpose(
                        ao_js[:, jb, q0 + qb * P:q0 + (qb + 1) * P],
                        aos[:, qb, :],
                    )

            # ---- filler queue: (deadline_slot, avail_slot, closure) -------
            filler = []

            def add_proj(tensor_sb, wsb, dst, jb, stl, dl, name):
                qf = make_proj_tile(tensor_sb, wsb, dst, jb, stl, name)
                filler.append([dl - 1, 0, lambda: qf(0)])
                filler.append([dl, 0, lambda: qf(1)])

            for s4 in range(1, 4):     # K jb0 stl 1..3, before task0 kb 4s
                add_proj(xk_sb, wk_sb, kt_sb, 0, s4, 4 * s4, f"k0{s4}")
            for k in range(KB):        # vjob k before av(0,k) at slot 16+k
                filler.append([16 + k, 0, (lambda kk: lambda: vjob(kk))(k)])
            for s4 in range(4):        # K jb1 before task2 kb 4s
                add_proj(xk_sb, wk_sb, kt_sb, 1, s4, 32 + 4 * s4, f"k1{s4}")
            for s4 in range(2):        # Q jb1 stl 0,1 before task 2
                add_proj(xq_sb, wq_sb, qt_sb, 1, s4, 31, f"q1{s4}")
            for s4 in range(2, 4):     # Q jb0 stl 2,3 before task 4
                add_proj(xq_sb, wq_sb, qt_sb, 0, s4, 63, f"q0{s4}")
            for s4 in range(2, 4):     # Q jb1 stl 2,3 before task 6
                add_proj(xq_sb, wq_sb, qt_sb, 1, s4, 95, f"q1{s4}")
            for eb in range(EB):       # oproj qh0 after transpose(0,1)@80
                for u in range(4):
                    filler.append([126, 81,
                                   (lambda e, uu: lambda: oproj_unit(0, e, uu))(eb, u)])
            for eb in range(4):        # oproj qh1 jb0-pass after transp(1,0)
                for u in range(4):     # (only 4 ebs: ostage pool depth)
                    filler.append([127, 113,
                                   (lambda e, uu: lambda: oproj_half(1, e, uu, 0))(eb, u)])

            filler.sort(key=lambda f: f[0])

            def drain_filler(slot, budget):
                done = 0
                for f in filler:
                    if done >= budget:
                        break
                    if f[1] <= slot:
                        f[2]()
                        f[0] = -1000
                        done += 1
                filler[:] = [f for f in filler if f[0] != -1000]
                return done

            # ---- phase 1: Q jb0 stl0/1 + K jb0 stl0 -----------------------
            for nm, (xsb, wsb, dst, jb, stl) in [
                ("ph_k00", (xk_sb, wk_sb, kt_sb, 0, 0)),
                ("ph_q00", (xq_sb, wq_sb, qt_sb, 0, 0)),
                ("ph_q01", (xq_sb, wq_sb, qt_sb, 0, 1)),
            ]:
                qf = make_proj_tile(xsb, wsb, dst, jb, stl, nm)
                qf(0)
                qf(1)

            # ---- slot machine ---------------------------------------------
            TASKS = [(qh, h) for qh in range(2) for h in range(HL)]
            oacc_of = {}

            def emit_scores_exp(ti, kb, n):
                qh, h = TASKS[ti]
                q0 = qh * QW + n * 512
                jb = h // 2
                off = DH * (h % 2)
                sc = psc.tile([P, 512], F32, tag="sc")
                nc.tensor.matmul(
                    sc[:],
                    kt_sb[off:off + DH, jb, kb * P:(kb + 1) * P],
                    qt_sb[off:off + DH, jb, q0:q0 + 512],
                    start=True,
                    stop=True,
                )
                ex = expp.tile([P, 512], F16, tag="ex", name=f"ex{ti}_{kb}_{n}")
                if (kb * 2 + n) % POOL_EVERY == 1 and kb < KB - 1:
                    stg = stgp.tile([P, 512], F32, tag="stg")
                    nc.vector.tensor_copy(stg[:], sc[:])
                    nc.gpsimd.tensor_tensor(ex[:], base2[:, 0:512],
                                            stg[:], op=ALU.pow)
                else:
                    nc.scalar.activation(ex[:], sc[:], AF.Exp, scale=LN2)
                return ex

            def emit_attnv(ti, kb, n, ex):
                qh, h = TASKS[ti]
                oacc = oacc_of[ti][n]
                for q4 in range(4):
                    # start=False always: start=True clears the whole PSUM
                    # bank, destroying the other q4 regions' partial sums.
                    # The bank is zeroed once by memzero at tile alloc.
                    nc.tensor.matmul(
                        oacc[:, q4 * (DH + 1):(q4 + 1) * (DH + 1)],
                        ex[:, q4 * P:(q4 + 1) * P],
                        v_sb[:, kb, h, :],
                        start=False,
                        stop=(kb == KB - 1),
                        skip_group_check=True,
                    )

            ex_of = {}
            for ti in range(8):
                # task top: norm(ti-2), transposes, oacc alloc
                if ti >= 2:
                    normalize(ti - 2, oacc_of[ti - 2])
                if ti == 3:
                    transpose_jb(0, 0)
                elif ti == 5:
                    transpose_jb(0, 1)
                elif ti == 7:
                    transpose_jb(1, 0)
                oacc_of[ti] = [
                    poacc.tile([P, 4 * (DH + 1)], F32, tag="oacc",
                               name=f"oa{ti}_{g}")
                    for g in range(2)
                ]
                for g in range(2):
                    nc.vector.memzero(oacc_of[ti][g][:])
                for kb in range(KB):
                    slot = ti * KB + kb
                    # filler and attnV (no psc dependency) first, so psc
                    # backpressure on scores doesn't starve the PE
                    drain_filler(slot, 2)
                    if ti >= 1:
                        for n in range(2):
                            emit_attnv(ti - 1, kb, n, ex_of.pop((ti - 1, kb, n)))
                    if ti == 7 and kb >= 6:
                        for n in range(2):
                            emit_attnv(7, kb - 6, n, ex_of.pop((7, kb - 6, n)))
                    for n in range(2):
                        ex_of[(ti, kb, n)] = emit_scores_exp(ti, kb, n)

            # ---- tail -----------------------------------------------------
            for kb in range(KB - 6, KB):
                for n in range(2):
                    emit_attnv(7, kb, n, ex_of.pop((7, kb, n)))
            drain_filler(200, 100)
            normalize(6, oacc_of[6])
            normalize(7, oacc_of[7])
            transpose_jb(1, 1)
            # interleave the DVE-add stream (split ebs, paux psum) with a
            # pair-fused ACT-evac stream (ebs 4-7, reusing the freed psc
            # banks, one 512-wide evac per unit pair) so both evac engines
            # and psum pools drain in parallel
            def oproj_pair(eb, up):
                q0 = QW
                po = psc.tile([P, 512], F32, tag="sc", name=f"pp2_{eb}_{up}")
                for half in range(2):
                    u = up * 2 + half
                    s0 = q0 + u * 256
                    for jb in range(2):
                        nc.tensor.matmul(
                            po[:, half * 256:half * 256 + 256],
                            wo_sb[:, jb, eb * P:(eb + 1) * P],
                            ao_js[:, jb, s0:s0 + 256],
                            start=(jb == 0),
                            stop=(jb == 1),
                        )
                ob = ob_group.setdefault(
                    (1, eb), opool.tile([P, QW], F16, tag="ob",
                                        name=f"ob1_{eb}"))
                nc.scalar.copy(ob[:, up * 512:(up + 1) * 512], po[:])
                nc.sync.dma_start(
                    out=out_t[eb][:, q0 + up * 512:q0 + (up + 1) * 512],
                    in_=ob[:, up * 512:(up + 1) * 512])

            for i in range(8):
                oproj_pair(4 + i // 2, i % 2)
                oproj_half(1, i // 2, (i % 2) * 2, 1)
                oproj_half(1, i // 2, (i % 2) * 2 + 1, 1)
            if DEBUG_TAPS:
                nc.sync.dma_start(out=dbg_qt.ap(), in_=qt_sb[:])
                nc.sync.dma_start(out=dbg_kt.ap(), in_=kt_sb[:])
                nc.sync.dma_start(out=dbg_v.ap(), in_=v_sb[:])
                nc.sync.dma_start(out=dbg_ao.ap(), in_=ao_js[:])

    nc.finalize()
    return nc


_NC_CACHE = None


def _get_nc():
    global _NC_CACHE
    if _NC_CACHE is None:
        _NC_CACHE = build_nc()
    return _NC_CACHE


def make_in_maps(query, key, value, Wq, Wk, Wv, Wo):
    """Build the 8 per-core input dicts from the full tensors (p-major)."""
    query = np.asarray(query, np.float32)
    key = np.asarray(key, np.float32)
    value = np.asarray(value, np.float32)
    Wq = np.asarray(Wq, np.float32)
    Wk = np.asarray(Wk, np.float32)
    Wv = np.asarray(Wv, np.float32)
    Wo = np.asarray(Wo, np.float32)

    def pmajor(a2d, inner):  # [Drows, inner] -> [P, Drows//P, inner]
        return np.ascontiguousarray(
            a2d.reshape(-1, P, inner).transpose(1, 0, 2)
        )

    # scores in the exp2 domain: fold log2(e)/sqrt(dh) into Wq
    scale = np.float32(np.log2(np.e) / np.sqrt(DH))
    xs = {}
    for b in range(B):
        xq = pmajor(np.ascontiguousarray(query[b].T), S).astype(np.float16)
        xk = pmajor(np.ascontiguousarray(key[b].T), S).astype(np.float16)
        xv3 = pmajor(np.ascontiguousarray(value[b].T), S).astype(np.float16)
        # xv: [P, dc, s] -> [P, g, sb4, dc*128] (s-block-major, groups of 4)
        xv = np.ascontiguousarray(
            xv3.reshape(P, KD, KB, P).transpose(0, 2, 1, 3).reshape(P, 4, -1)
        )
        xs[b] = {"xq": xq, "xk": xk, "xv": xv}
    ws = {}
    for hg in range(4):
        sl = slice(hg * J, (hg + 1) * J)
        wo_t = np.ascontiguousarray(Wo[:, sl].T)  # [256, 1024]
        ws[hg] = {
            "wq": pmajor(np.ascontiguousarray(Wq[sl].T * scale), J).astype(np.float16),
            "wk": pmajor(np.ascontiguousarray(Wk[sl].T), J).astype(np.float16),
            "wv": pmajor(np.ascontiguousarray(Wv[sl].T), J).astype(np.float16),
            "wo": np.ascontiguousarray(
                wo_t.reshape(2, P, D).transpose(1, 0, 2)
            ).astype(np.float16),
        }
    in_maps = []
    for c in range(NCORES):
        b, hg = c // 4, c % 4
        m = {}
        m.update(xs[b])
        m.update(ws[hg])
        in_maps.append(m)
    return in_maps


def assemble(results, bo):
    """Sum the 4 per-core partials per batch, add bo."""
    bo = np.asarray(bo, np.float32)
    out = np.zeros((B, S, D), np.float32)
    for c in range(NCORES):
        b = c // 4
        part = results[c]["out_t"].astype(np.float32).reshape(D, S).T
        out[b] += part
    out += bo[None, None, :]
    return out


def kernel(query, key, value, Wq, Wk, Wv, Wo, bo):
    import os
    import time

    # helps recover wedged NeuronCores between runs
    os.environ.setdefault("NEURON_RT_RESET_CORES", "1")
    from concourse.bass_utils import run_bass_kernel_spmd

    nc = _get_nc()
    in_maps = make_in_maps(query, key, value, Wq, Wk, Wv, Wo)
    last_exc = None
    for attempt in range(3):
        try:
            res = run_bass_kernel_spmd(nc, in_maps, list(range(NCORES)))
            return assemble(res.results, bo)
        except Exception as e:  # transient NRT_EXEC_UNIT_UNRECOVERABLE etc.
            last_exc = e
            time.sleep(2.0)
    raise last_exc


# revision 123
# speedup vs baseline: 1.0013x; 1.0010x over previous
"""Multi-head attention (B=2, S=2048, D=1024, H=16, Dh=64) on 8 TRN2 cores.

Sharding: data-parallel over batch (2) x tensor-parallel over heads (16 -> 4
groups of 4). Core c handles batch c//4, heads [4*(c%4), 4*(c%4)+4).
Each core computes its partial output projection (Wo column slice); the host
sums the 4 per-core partials per batch (the "all-reduce") and adds bo.

v3 design (cost-model driven; PE cost = sum of matmul moving dims only):
  - Scores in the exp2 domain (log2e/sqrt(dh) folded into Wq); exp runs on
    ACT as Exp(scale=ln2) except every POOL_EVERY-th tile, which DVE stages
    PSUM->SBUF and GPSIMD computes as pow(2, x) -> fp16.
  - attn@V is d-moving: oacc[q, 65] += ex[k, qb].T @ (v|ones)[k, 65]; the
    ones column accumulates the softmax denominator. Normalization is a
    per-partition reciprocal multiply on DVE. fp16 [q, j] stage tiles are
    transposed to [j, q] by DMA xbar transposes for the output projection.
  - V is projected directly in [s, j] (lhsT = x_v s-block), no transposes.
  - Slot machine: task i's scores/exp run one task AHEAD of task i-1's
    attn@V, hiding the whole exp latency chain; Q/K projection tiles (by
    (jb, stl)) and V s-blocks are just-in-time filler quanta with deadlines,
    drained EDF into the PE slack of each slot. x streams stl-granular so
    the first head starts after only 3 projection tiles.
"""

import numpy as np
from contextlib import ExitStack

import concourse.bass as bass
from concourse import bacc
import concourse.mybir as mybir
import concourse.tile as tile

F32 = mybir.dt.float32
F16 = mybir.dt.float16
AF = mybir.ActivationFunctionType
ALU = mybir.AluOpType

B = 2
S = 2048
D = 1024
H = 16
DH = 64
NCORES = 8
HL = 4          # heads per core
J = HL * DH     # 256 local projection width
P = 128
KD = D // P     # 8 d-chunks (contraction steps)
KB = S // P     # 16 k-blocks
QW = 1024       # q columns per attention task
EB = D // P     # 8 e-blocks
LN2 = float(np.log(2.0))

POOL_EVERY = 5  # every POOL_EVERY-th exp tile -> DVE stage + gpsimd pow2
DEBUG_TAPS = False


def build_nc():
    nc = bacc.Bacc()

    xq = nc.dram_tensor("xq", [P, KD, S], F16, kind="ExternalInput")
    xk = nc.dram_tensor("xk", [P, KD, S], F16, kind="ExternalInput")
    xv = nc.dram_tensor("xv", [P, 4, 4 * KD * P], F16, kind="ExternalInput")
    wq = nc.dram_tensor("wq", [P, KD, J], F16, kind="ExternalInput")
    wk = nc.dram_tensor("wk", [P, KD, J], F16, kind="ExternalInput")
    wv = nc.dram_tensor("wv", [P, KD, J], F16, kind="ExternalInput")
    wo = nc.dram_tensor("wo", [P, 2, D], F16, kind="ExternalInput")
    out_t = nc.dram_tensor("out_t", [EB, P, S], F16, kind="ExternalOutput")
    if DEBUG_TAPS:
        dbg_qt = nc.dram_tensor("dbg_qt", [P, 2, S], F16, kind="ExternalOutput")
        dbg_kt = nc.dram_tensor("dbg_kt", [P, 2, S], F16, kind="ExternalOutput")
        dbg_v = nc.dram_tensor("dbg_v", [P, KB, HL, DH + 1], F16,
                               kind="ExternalOutput")
        dbg_ao = nc.dram_tensor("dbg_ao", [P, 2, S], F16, kind="ExternalOutput")

    with tile.TileContext(nc) as tc, ExitStack() as st:
        const = st.enter_context(tc.tile_pool(name="const", bufs=1))
        persist = st.enter_context(tc.tile_pool(name="persist", bufs=1))
        xvpool = st.enter_context(tc.tile_pool(name="xvp", bufs=3))

        wq_sb = const.tile([P, KD, J], F16, tag="wq")
        wk_sb = const.tile([P, KD, J], F16, tag="wk")
        wv_sb = const.tile([P, KD, J], F16, tag="wv")
        wo_sb = const.tile([P, 2, D], F16, tag="wo")
        base2 = const.tile([P, QW], F32, tag="base2")

        xq_sb = persist.tile([P, KD, S], F16, tag="xq")
        xk_sb = persist.tile([P, KD, S], F16, tag="xk")
        qt_sb = persist.tile([P, 2, S], F16, tag="qt")    # Q_T [256, 2048]
        kt_sb = persist.tile([P, 2, S], F16, tag="kt")    # K_T
        v_sb = persist.tile([P, KB, HL, DH + 1], F16, tag="v")  # V + ones
        ao_js = persist.tile([P, 2, S], F16, tag="ao")    # [j, q] for oproj

        # ---- DMA preamble (SP queue, stl-granular so JIT proj can start) --
        def dma_x(dst, src, stl, dchalf=None):
            d0, d1 = (0, KD) if dchalf is None else (dchalf * 4, dchalf * 4 + 4)
            nc.sync.dma_start(
                out=dst[:, d0:d1, stl * 512:(stl + 1) * 512],
                in_=src[:, d0:d1, stl * 512:(stl + 1) * 512])

        nc.sync.dma_start(out=wk_sb[:], in_=wk[:])
        dma_x(xk_sb, xk, 0)
        nc.sync.dma_start(out=wq_sb[:], in_=wq[:])
        dma_x(xq_sb, xq, 0)
        dma_x(xq_sb, xq, 1)
        dma_x(xk_sb, xk, 1)
        dma_x(xk_sb, xk, 2)
        dma_x(xk_sb, xk, 3)
        nc.sync.dma_start(out=wv_sb[:], in_=wv[:])
        xv_ch = []
        for g in range(4):
            c = xvpool.tile([P, 4, KD * P], F16, tag="xvc", name=f"xv{g}")
            nc.sync.dma_start(out=c[:], in_=xv[:, g, :])
            xv_ch.append(c)
        dma_x(xq_sb, xq, 2)
        dma_x(xq_sb, xq, 3)
        nc.sync.dma_start(out=wo_sb[:], in_=wo[:])
        nc.gpsimd.memset(base2[:], 2.0)
        nc.gpsimd.memset(v_sb[:, :, :, DH:DH + 1], 1.0)

        # ---- helpers ------------------------------------------------------
        with tc.tile_pool(name="psc", bufs=4, space="PSUM") as psc, \
             tc.tile_pool(name="poacc", bufs=2, space="PSUM") as poacc, \
             tc.tile_pool(name="paux", bufs=2, space="PSUM") as paux, \
             tc.tile_pool(name="expp", bufs=36) as expp, \
             tc.tile_pool(name="stgp", bufs=3) as stgp, \
             tc.tile_pool(name="aost", bufs=3) as aop, \
             tc.tile_pool(name="npool", bufs=4) as npool, \
             tc.tile_pool(name="ostage", bufs=6) as opool:

            # Q/K projection tile (tensor, jb, stl): PSUM [128 j, 512 s],
            # accumulated over 8 dc in 2 quanta of 4, evacuated to qt/kt.
            def make_proj_tile(xsb, wsb, dst, jb, stl, name):
                state = {"ps": None}

                def quantum(q):
                    if q == 0:
                        state["ps"] = paux.tile([P, 512], F32, tag="vp",
                                                name=f"pp_{name}")
                    ps = state["ps"]
                    for dc in range(q * 4, q * 4 + 4):
                        nc.tensor.matmul(
                            ps[:],
                            wsb[:, dc, jb * P:(jb + 1) * P],
                            xsb[:, dc, stl * 512:(stl + 1) * 512],
                            start=(dc == 0),
                            stop=(dc == KD - 1),
                        )
                    if q == 1:
                        nc.vector.tensor_copy(
                            dst[:, jb, stl * 512:(stl + 1) * 512], ps[:])

                return quantum

            # V s-block job: PSUM [128 s, 256 j] over 8 dc, one quantum.
            def vjob(sb):
                vp = paux.tile([P, J], F32, tag="vp", name=f"vp{sb}")
                xc = xv_ch[sb // 4]
                for dc in range(KD):
                    nc.tensor.matmul(
                        vp[:],
                        xc[:, sb % 4, dc * P:(dc + 1) * P],
                        wv_sb[:, dc, :],
                        start=(dc == 0),
                        stop=(dc == KD - 1),
                    )
                nc.vector.tensor_copy(v_sb[:, sb, :, 0:DH], vp[:])

            # oproj unit: 256 out cols, jb0+jb1 accumulate, fp16 stage+DMA.
            ob_group = {}

            def oproj_unit(qh, eb, u, pool=None, act_evac=False):
                q0 = qh * QW
                s0 = q0 + u * 256
                pool = pool if pool is not None else paux
                tag = "vp" if pool is paux else "oacc"
                po = pool.tile([P, J], F32, tag=tag, name=f"po{qh}_{eb}_{u}")
                for jb in range(2):
                    nc.tensor.matmul(
                        po[:, 0:256],
                        wo_sb[:, jb, eb * P:(eb + 1) * P],
                        ao_js[:, jb, s0:s0 + 256],
                        start=(jb == 0),
                        stop=(jb == 1),
                    )
                key = (qh, eb)
                if key not in ob_group:
                    ob_group[key] = opool.tile([P, QW], F16, tag="ob",
                                               name=f"ob{qh}_{eb}")
                ob = ob_group[key]
                if qh == 1 or act_evac:
                    nc.scalar.copy(ob[:, u * 256:(u + 1) * 256], po[:, 0:256])
                else:
                    nc.vector.tensor_copy(ob[:, u * 256:(u + 1) * 256],
                                          po[:, 0:256])
                if u in (1, 3):
                    h0 = (u - 1) * 256
                    nc.sync.dma_start(out=out_t[eb][:, q0 + h0:q0 + h0 + 512],
                                      in_=ob[:, h0:h0 + 512])

            # split-jb variant for the last q-half: jb0 partials run during
            # task 7 (only needs transpose(1,0)); the jb1 pass + add is the
            # short tail gated on the final norm.
            def oproj_half(qh, eb, u, jb):
                q0 = qh * QW
                s0 = q0 + u * 256
                po = paux.tile([P, J], F32, tag="vp",
                               name=f"ph{qh}_{eb}_{u}_{jb}")
                nc.tensor.matmul(
                    po[:, 0:256],
                    wo_sb[:, jb, eb * P:(eb + 1) * P],
                    ao_js[:, jb, s0:s0 + 256],
                    start=True,
                    stop=True,
                )
                key = (qh, eb)
                if key not in ob_group:
                    ob_group[key] = opool.tile([P, QW], F16, tag="ob",
                                               name=f"ob{qh}_{eb}")
                ob = ob_group[key]
                dst = ob[:, u * 256:(u + 1) * 256]
                if jb == 0:
                    # runs inside task 7 where ACT is still on exp: use DVE
                    nc.vector.tensor_copy(dst, po[:, 0:256])
                else:
                    nc.vector.tensor_add(dst, dst, po[:, 0:256])
                    if u in (1, 3):
                        h0 = (u - 1) * 256
                        nc.sync.dma_start(
                            out=out_t[eb][:, q0 + h0:q0 + h0 + 512],
                            in_=ob[:, h0:h0 + 512])

            ao_stage = {}

            def normalize(ti, oaccs, transp_cb=None):
                qh, h = TASKS[ti]
                jb = h // 2
                off = DH * (h % 2)
                key = (qh, jb)
                if key not in ao_stage:
                    ao_stage[key] = aop.tile([P, 8, P], F16, tag="aos",
                                             name=f"aos{qh}_{jb}")
                aos = ao_stage[key]
                for g in range(2):
                    rc = npool.tile([P, 4], F32, tag="rc")
                    with tc.high_priority(offset=400):
                        nc.vector.reciprocal_approx_fast(
                            rc[:], oaccs[g][:, DH::DH + 1])
                    for q4 in range(4):
                        qb = g * 4 + q4
                        src_ap = oaccs[g][:, q4 * (DH + 1):q4 * (DH + 1) + DH]
                        dst_ap = aos[:, qb, off:off + DH]
                        nc.vector.tensor_scalar(
                            dst_ap, src_ap, rc[:, q4:q4 + 1], None,
                            op0=ALU.mult)
                    if transp_cb is not None:
                        transp_cb(g)  # transpose qb-group as soon as normed

            def transpose_jb(qh, jb, qbs=range(8)):
                aos = ao_stage[(qh, jb)]
                q0 = qh * QW
                for qb in qbs:
                    nc.sync.dma_start_transpose(
                        ao_js[:, jb, q0 + qb * P:q0 + (qb + 1) * P],
                        aos[:, qb, :],
                    )

            # ---- filler queue: (deadline_slot, avail_slot, closure) -------
            filler = []

            def add_proj(tensor_sb, wsb, dst, jb, stl, dl, name):
                qf = make_proj_tile(tensor_sb, wsb, dst, jb, stl, name)
                filler.append([dl - 1, 0, lambda: qf(0)])
                filler.append([dl, 0, lambda: qf(1)])

            for s4 in range(1, 4):     # K jb0 stl 1..3, before task0 kb 4s
                add_proj(xk_sb, wk_sb, kt_sb, 0, s4, 4 * s4, f"k0{s4}")
            for k in range(KB):        # vjob k before av(0,k) at slot 16+k
                filler.append([16 + k, 0, (lambda kk: lambda: vjob(kk))(k)])
            for s4 in range(4):        # K jb1 before task2 kb 4s
                add_proj(xk_sb, wk_sb, kt_sb, 1, s4, 32 + 4 * s4, f"k1{s4}")
            for s4 in range(2):        # Q jb1 stl 0,1 before task 2
                add_proj(xq_sb, wq_sb, qt_sb, 1, s4, 31, f"q1{s4}")
            for s4 in range(2, 4):     # Q jb0 stl 2,3 before task 4
                add_proj(xq_sb, wq_sb, qt_sb, 0, s4, 63, f"q0{s4}")
            for s4 in range(2, 4):     # Q jb1 stl 2,3 before task 6
                add_proj(xq_sb, wq_sb, qt_sb, 1, s4, 95, f"q1{s4}")
            for eb in range(EB):       # oproj qh0 after transpose(0,1)@80
                for u in range(4):
                    filler.append([126, 81,
                                   (lambda e, uu: lambda: oproj_unit(0, e, uu))(eb, u)])
            for eb in range(4):        # oproj qh1 jb0-pass after transp(1,0)
                for u in range(4):     # (only 4 ebs: ostage pool depth)
                    filler.append([127, 113,
                                   (lambda e, uu: lambda: oproj_half(1, e, uu, 0))(eb, u)])

            filler.sort(key=lambda f: f[0])

            def drain_filler(slot, budget):
                done = 0
                for f in filler:
                    if done >= budget:
                        break
                    if f[1] <= slot:
                        f[2]()
                        f[0] = -1000
                        done += 1
                filler[:] = [f for f in filler if f[0] != -1000]
                return done

            # ---- phase 1: Q jb0 stl0/1 + K jb0 stl0 -----------------------
            for nm, (xsb, wsb, dst, jb, stl) in [
                ("ph_k00", (xk_sb, wk_sb, kt_sb, 0, 0)),
                ("ph_q00", (xq_sb, wq_sb, qt_sb, 0, 0)),
                ("ph_q01", (xq_sb, wq_sb, qt_sb, 0, 1)),
            ]:
                qf = make_proj_tile(xsb, wsb, dst, jb, stl, nm)
                qf(0)
                qf(1)

            # ---- slot machine ---------------------------------------------
            TASKS = [(qh, h) for qh in range(2) for h in range(HL)]
            oacc_of = {}

            def emit_scores_exp(ti, kb, n):
                qh, h = TASKS[ti]
                q0 = qh * QW + n * 512
                jb = h // 2
                off = DH * (h % 2)
                sc = psc.tile([P, 512], F32, tag="sc")
                nc.tensor.matmul(
                    sc[:],
                    kt_sb[off:off + DH, jb, kb * P:(kb + 1) * P],
                    qt_sb[off:off + DH, jb, q0:q0 + 512],
                    start=True,
                    stop=True,
                )
                ex = expp.tile([P, 512], F16, tag="ex", name=f"ex{ti}_{kb}_{n}")
                if (kb * 2 + n) % POOL_EVERY == 1 and kb < KB - 1:
                    stg = stgp.tile([P, 512], F32, tag="stg")
                    nc.vector.tensor_copy(stg[:], sc[:])
                    nc.gpsimd.tensor_tensor(ex[:], base2[:, 0:512],
                                            stg[:], op=ALU.pow)
                else:
                    nc.scalar.activation(ex[:], sc[:], AF.Exp, scale=LN2)
                return ex

            def emit_attnv(ti, kb, n, ex):
                qh, h = TASKS[ti]
                oacc = oacc_of[ti][n]
                for q4 in range(4):
                    # start=False always: start=True clears the whole PSUM
                    # bank, destroying the other q4 regions' partial sums.
                    # The bank is zeroed once by memzero at tile alloc.
                    nc.tensor.matmul(
                        oacc[:, q4 * (DH + 1):(q4 + 1) * (DH + 1)],
                        ex[:, q4 * P:(q4 + 1) * P],
                        v_sb[:, kb, h, :],
                        start=False,
                        stop=(kb == KB - 1),
                        skip_group_check=True,
                    )

            ex_of = {}
            for ti in range(8):
                # task top: norm(ti-2), transposes, oacc alloc
                if ti >= 2:
                    normalize(ti - 2, oacc_of[ti - 2])
                if ti == 3:
                    transpose_jb(0, 0)
                elif ti == 5:
                    transpose_jb(0, 1)
                elif ti == 7:
                    transpose_jb(1, 0)
                oacc_of[ti] = [
                    poacc.tile([P, 4 * (DH + 1)], F32, tag="oacc",
                               name=f"oa{ti}_{g}")
                    for g in range(2)
                ]
                for g in range(2):
                    nc.vector.memzero(oacc_of[ti][g][:])
                for kb in range(KB):
                    slot = ti * KB + kb
                    # filler and attnV (no psc dependency) first, so psc
                    # backpressure on scores doesn't starve the PE
                    drain_filler(slot, 2)
                    if ti >= 1:
                        for n in range(2):
                            emit_attnv(ti - 1, kb, n, ex_of.pop((ti - 1, kb, n)))
                    if ti == 7 and kb >= 6:
                        for n in range(2):
                            emit_attnv(7, kb - 6, n, ex_of.pop((7, kb - 6, n)))
                    for n in range(2):
                        ex_of[(ti, kb, n)] = emit_scores_exp(ti, kb, n)

            # ---- tail -----------------------------------------------------
            for kb in range(KB - 4, KB):
                for n in range(2):
                    emit_attnv(7, kb, n, ex_of.pop((7, kb, n)))
            drain_filler(200, 100)
            normalize(6, oacc_of[6])
            normalize(7, oacc_of[7])
            transpose_jb(1, 1)
            # interleave the DVE-add stream (split ebs, paux psum) with a
            # pair-fused ACT-evac stream (ebs 4-7, reusing the freed psc
            # banks, one 512-wide evac per unit pair) so both evac engines
            # and psum pools drain in parallel
            def oproj_pair(eb, up):
                q0 = QW
                po = psc.tile([P, 512], F32, tag="sc", name=f"pp2_{eb}_{up}")
                for half in range(2):
                    u = up * 2 + half
                    s0 = q0 + u * 256
                    for jb in range(2):
                        nc.tensor.matmul(
                            po[:, half * 256:half * 256 + 256],
                            wo_sb[:, jb, eb * P:(eb + 1) * P],
                            ao_js[:, jb, s0:s0 + 256],
                            start=(jb == 0),
                            stop=(jb == 1),
                        )
                ob = ob_group.setdefault(
                    (1, eb), opool.tile([P, QW], F16, tag="ob",
                                        name=f"ob1_{eb}"))
                nc.scalar.copy(ob[:, up * 512:(up + 1) * 512], po[:])
                nc.sync.dma_start(
                    out=out_t[eb][:, q0 + up * 512:q0 + (up + 1) * 512],
                    in_=ob[:, up * 512:(up + 1) * 512])

            for i in range(8):
                oproj_pair(4 + i // 2, i % 2)
                oproj_half(1, i // 2, (i % 2) * 2, 1)
                oproj_half(1, i // 2, (i % 2) * 2 + 1, 1)
            if DEBUG_TAPS:
                nc.sync.dma_start(out=dbg_qt.ap(), in_=qt_sb[:])
                nc.sync.dma_start(out=dbg_kt.ap(), in_=kt_sb[:])
                nc.sync.dma_start(out=dbg_v.ap(), in_=v_sb[:])
                nc.sync.dma_start(out=dbg_ao.ap(), in_=ao_js[:])

    nc.finalize()
    return nc


_NC_CACHE = None


def _get_nc():
    global _NC_CACHE
    if _NC_CACHE is None:
        _NC_CACHE = build_nc()
    return _NC_CACHE


def make_in_maps(query, key, value, Wq, Wk, Wv, Wo):
    """Build the 8 per-core input dicts from the full tensors (p-major)."""
    query = np.asarray(query, np.float32)
    key = np.asarray(key, np.float32)
    value = np.asarray(value, np.float32)
    Wq = np.asarray(Wq, np.float32)
    Wk = np.asarray(Wk, np.float32)
    Wv = np.asarray(Wv, np.float32)
    Wo = np.asarray(Wo, np.float32)

    def pmajor(a2d, inner):  # [Drows, inner] -> [P, Drows//P, inner]
        return np.ascontiguousarray(
            a2d.reshape(-1, P, inner).transpose(1, 0, 2)
        )

    # scores in the exp2 domain: fold log2(e)/sqrt(dh) into Wq
    scale = np.float32(np.log2(np.e) / np.sqrt(DH))
    xs = {}
    for b in range(B):
        xq = pmajor(np.ascontiguousarray(query[b].T), S).astype(np.float16)
        xk = pmajor(np.ascontiguousarray(key[b].T), S).astype(np.float16)
        xv3 = pmajor(np.ascontiguousarray(value[b].T), S).astype(np.float16)
        # xv: [P, dc, s] -> [P, g, sb4, dc*128] (s-block-major, groups of 4)
        xv = np.ascontiguousarray(
            xv3.reshape(P, KD, KB, P).transpose(0, 2, 1, 3).reshape(P, 4, -1)
        )
        xs[b] = {"xq": xq, "xk": xk, "xv": xv}
    ws = {}
    for hg in range(4):
        sl = slice(hg * J, (hg + 1) * J)
        wo_t = np.ascontiguousarray(Wo[:, sl].T)  # [256, 1024]
        ws[hg] = {
            "wq": pmajor(np.ascontiguousarray(Wq[sl].T * scale), J).astype(np.float16),
            "wk": pmajor(np.ascontiguousarray(Wk[sl].T), J).astype(np.float16),
            "wv": pmajor(np.ascontiguousarray(Wv[sl].T), J).astype(np.float16),
            "wo": np.ascontiguousarray(
                wo_t.reshape(2, P, D).transpose(1, 0, 2)
            ).astype(np.float16),
        }
    in_maps = []
    for c in range(NCORES):
        b, hg = c // 4, c % 4
        m = {}
        m.update(xs[b])
        m.update(ws[hg])
        in_maps.append(m)
    return in_maps


def assemble(results, bo):
    """Sum the 4 per-core partials per batch, add bo."""
    bo = np.asarray(bo, np.float32)
    out = np.zeros((B, S, D), np.float32)
    for c in range(NCORES):
        b = c // 4
        part = results[c]["out_t"].astype(np.float32).reshape(D, S).T
        out[b] += part
    out += bo[None, None, :]
    return out


def kernel(query, key, value, Wq, Wk, Wv, Wo, bo):
    import os
    import time

    # helps recover wedged NeuronCores between runs
    os.environ.setdefault("NEURON_RT_RESET_CORES", "1")
    from concourse.bass_utils import run_bass_kernel_spmd

    nc = _get_nc()
    in_maps = make_in_maps(query, key, value, Wq, Wk, Wv, Wo)
    last_exc = None
    for attempt in range(3):
        try:
            res = run_bass_kernel_spmd(nc, in_maps, list(range(NCORES)))
            return assemble(res.results, bo)
        except Exception as e:  # transient NRT_EXEC_UNIT_UNRECOVERABLE etc.
            last_exc = e
            time.sleep(2.0)
    raise last_exc
